# revision 25
# baseline (speedup 1.0000x reference)
"""Trainium2 Bass kernel for nn_EvMLP (segment_reduce EvNorm + invariant MLP).

Self-contained: hardcodes shapes/sharding. Accepts FULL inputs, returns FULL
output; shards the node dim N across 8 NeuronCores (pure data parallel).

v3 design (on top of the bf16 v2 baseline):
  - x10T supplied pre-transposed by the HOST (plain contiguous DMA load;
    the HW DMA-transpose of [R,128] made 256B packets and throttled DMA)
  - all eq squares on ACT (one Square op per macro); GP keeps only x2
  - x11 written into a 128-wide padded tile; x11T produced by an
    SBUF->SBUF DMA transpose (PE transposes + ACT PSUM-copies eliminated);
    w1b/W12b zero-padded to [128,128] so junk partitions multiply by 0
  - LN stats batched per macro: Q1/Q2 are [128, nb*128] f32 PSUM tiles
    (2 banks), seed + NR custom ops run at FD=1024 (overhead amortized)
  - b3 rank-1 matmul dropped (b3c==0 for this problem; np fallback guards)
"""
import sys

sys.path.insert(0, "/opt/trn_rl_repo")

import numpy as np

import concourse.bass as bass
import concourse.bacc as bacc
import concourse.tile as tile
from concourse import mybir
from concourse.bass_utils import run_bass_kernel_spmd

f32 = mybir.dt.float32
i32 = mybir.dt.int32
bf16 = mybir.dt.bfloat16

# ---------------------------------------------------------------- constants --
N = 100000
DIM = 592
N_INV = 128
N_EQ_CH = 112
N_EQ = 464
EPS = 1e-5
N_CORES = 8
BLOCKS_PER_CORE = 98                      # 98*128 = 12544 rows/core
ROWS_PER_CORE = BLOCKS_PER_CORE * 128
NPAD = N_CORES * ROWS_PER_CORE            # 100352
MACROS = [8] * 12 + [2]                   # blocks per macro-tile (sum 98)
CHUNK_BLOCKS = 4                          # rows per MLP chunk = 512
MAGIC = 0x5F3759DF
MAGICF = float(MAGIC)

# segment groups: (n_channels, width, eq column offset, channel offset)
SEGS = [(64, 3, 0, 0), (32, 5, 192, 64), (16, 7, 352, 96)]

_EXPECTED_REP = np.concatenate(
    [np.repeat(np.arange(m) + off, 2 * l + 1)
     for l, (m, off) in enumerate([(128, 0), (64, 128), (32, 192), (16, 224)])]
)

# ------------------------------------------------------------- custom DVE op --
from concourse.dve_spec import Spec, Src0, Src1, C0, C1, C2, lower
from concourse.dve_uop import DveOpSpec
import concourse.dve_ops as dve_ops
from concourse.dve_ops import DveOp

# Newton rsqrt step: out = y*(C1 - C0*((v+C2)*y*y));  in0=v, in1=y
_nr_body = Src1 * (C1 - ((Src0 + C2) * (Src1 * Src1)) * C0)


def _nr_ref(in0, in1, s0, s1, imm2):
    y = in1.astype(np.float32)
    v = in0.astype(np.float32)
    return (y * (np.float32(s1) - ((v + np.float32(imm2)) * y * y) * np.float32(s0))
            ).astype(np.float32)


def _register(name, spec):
    if name in dve_ops._SUB_OPCODE_FOR_NAME:
        for op in dve_ops.OPS:
            if op.name == name:
                return op
    shas = {}
    row = 1 + len(dve_ops.OPS)
    for ver in ("v3", "v4"):
        s = DveOpSpec(name=name, opcode=row, uops=lower(spec, ver=ver), rd1_en=True)
        shas[ver] = s.sha(ver)
    op = DveOp(name, spec, subdim=False, uops_sha=shas)
    dve_ops.OPS.append(op)
    dve_ops._SUB_OPCODE_FOR_NAME[name] = row
    dve_ops.CUSTOM_DVE_SPECS[name] = spec
    return op


RSQRT_NR = _register("ANT_RSQRT_NR2", Spec(body=_nr_body, reference=_nr_ref))


def _make_mulsub1():
    from concourse.dve_spec import One
    return _register(
        "ANT_MUL_SUB1",
        Spec(
            body=(Src0 * Src1) - One,
            reference=lambda in0, in1, s0, s1, imm2: (
                in0.astype(np.float32) * in1 - np.float32(1.0)
            ).astype(np.float32),
        ),
    )


MUL_SUB1 = _make_mulsub1()


# ------------------------------------------------------------ kernel builder --
def _build_nc():
    nc = bacc.Bacc()

    x = nc.dram_tensor("x", [ROWS_PER_CORE, DIM], bf16, kind="ExternalInput")
    x10t_d = nc.dram_tensor("x10t", [128, ROWS_PER_CORE], bf16, kind="ExternalInput")
    out = nc.dram_tensor("out", [ROWS_PER_CORE, DIM], bf16, kind="ExternalOutput")
    w1a_d = nc.dram_tensor("w1a", [128, 128], bf16, kind="ExternalInput")
    w1b_d = nc.dram_tensor("w1b", [112, 128], bf16, kind="ExternalInput")
    w12a_d = nc.dram_tensor("w12a", [128, 128], bf16, kind="ExternalInput")
    w12b_d = nc.dram_tensor("w12b", [112, 128], bf16, kind="ExternalInput")
    w3_d = nc.dram_tensor("w3p", [128, 128], bf16, kind="ExternalInput")
    cmat_d = nc.dram_tensor("cmat", [128, 128], bf16, kind="ExternalInput")
    onesd_d = nc.dram_tensor("onesd", [128, 128], bf16, kind="ExternalInput")
    ident_d = nc.dram_tensor("ident", [128, 128], bf16, kind="ExternalInput")
    b2_d = nc.dram_tensor("b2c", [128, 1], f32, kind="ExternalInput")

    # extra float consts used as activation bias (register like Bass.__init__)
    for _v in (MAGICF, float(EPS), 1.0):
        _t = nc.alloc_sbuf_tensor(f"const-f32-{_v}", [128, 1], f32)
        nc.gpsimd.memset(_t.ap(), _v)
        nc.const_aps.aps[(f32, _v)] = _t.ap()
    nc.all_engine_barrier()

    AF = mybir.ActivationFunctionType
    ALU = mybir.AluOpType
    AX = mybir.AxisListType

    from contextlib import ExitStack

    with tile.TileContext(nc) as tc:
        with ExitStack() as ctx:
            wpool = ctx.enter_context(tc.tile_pool(name="w", bufs=1))
            xpool = ctx.enter_context(tc.tile_pool(name="xp", bufs=3))
            opool = ctx.enter_context(tc.tile_pool(name="op", bufs=3))
            epool = ctx.enter_context(tc.tile_pool(name="ep", bufs=3))
            spool = ctx.enter_context(tc.tile_pool(name="sp", bufs=4))
            cpool = ctx.enter_context(tc.tile_pool(name="cp", bufs=3))
            tpool = ctx.enter_context(tc.tile_pool(name="tp", bufs=2))
            ps_mm = ctx.enter_context(tc.tile_pool(name="pmm", bufs=7, space="PSUM"))
            ps_tp = ctx.enter_context(tc.tile_pool(name="ptp", bufs=1, space="PSUM"))

            def wtile(name, dram, shape, dtype):
                t = wpool.tile(shape, dtype, tag=name)
                nc.sync.dma_start(out=t, in_=dram[:, :])
                return t

            w1a = wtile("w1a", w1a_d, [128, 128], bf16)
            w1b = wtile("w1b", w1b_d, [112, 128], bf16)
            w12a = wtile("w12a", w12a_d, [128, 128], bf16)
            w12b = wtile("w12b", w12b_d, [112, 128], bf16)
            w3p = wtile("w3p", w3_d, [128, 128], bf16)
            cmat = wtile("cmat", cmat_d, [128, 128], bf16)
            onesd = wtile("onesd", onesd_d, [128, 128], bf16)
            ident = wtile("ident", ident_d, [128, 128], bf16)
            b2c = wtile("b2c", b2_d, [128, 1], f32)

            flat3 = lambda ap: ap.rearrange("p a b -> p (a b)")

            def mm512(out_ap, lhsT, rhs, start, stop, skip=False):
                F = rhs.shape[-1]
                for f0 in range(0, F, 512):
                    f1 = min(f0 + 512, F)
                    nc.tensor.matmul(
                        out_ap[:, f0:f1], lhsT, rhs[:, f0:f1],
                        start=start, stop=stop, skip_group_check=skip,
                    )

            def issue_load(row0, nb):
                """Prefetch macro tiles (issued one iteration early)."""
                R_rows = nb * 128
                xev = x[row0 : row0 + R_rows, N_INV:DIM].rearrange(
                    "(b p) d -> p b d", p=128
                )
                ov = out[row0 : row0 + R_rows, :].rearrange("(b p) d -> p b d", p=128)

                Xe = xpool.tile([128, nb, N_EQ], bf16, tag="Xe")
                nc.sync.dma_start(out=Xe, in_=xev)
                X10T = xpool.tile([128, R_rows], bf16, tag="X10T")
                nc.sync.dma_start(out=X10T, in_=x10t_d[:, row0 : row0 + R_rows])
                return dict(nb=nb, ov=ov, Xe=Xe, X10T=X10T)

            def issue_eq(t):
                """eq chain through x11 (inputs prefetched last iteration)."""
                nb, Xe = t["nb"], t["Xe"]
                eq2 = epool.tile([128, nb, N_EQ], bf16, tag="eq2")
                for (nch, w, eqoff, choff) in SEGS:
                    e0, e1 = eqoff, eqoff + nch * w
                    nc.scalar.activation(
                        out=eq2[:, :, e0:e1], in_=Xe[:, :, e0:e1], func=AF.Square)

                sumsq = spool.tile([128, nb, N_EQ_CH], f32, tag="sumsq")
                for (nch, w, eqoff, choff) in SEGS:
                    nc.vector.reduce_sum(
                        out=sumsq[:, :, choff : choff + nch],
                        in_=eq2[:, :, eqoff : eqoff + nch * w].rearrange(
                            "p b (c t) -> p b c t", t=w
                        ),
                        axis=AX.X,
                    )

                s1 = spool.tile([128, nb, N_EQ_CH], f32, tag="s1")
                nc.scalar.activation(out=s1, in_=sumsq, func=AF.Identity, bias=1.0)
                seedb = spool.tile([128, nb, N_EQ_CH], i32, tag="seedb")
                nc.scalar.activation(
                    out=seedb, in_=s1.bitcast(i32), func=AF.Identity,
                    scale=-0.5, bias=MAGICF,
                )
                r = spool.tile([128, nb, N_EQ_CH], f32, tag="r")
                nc.vector._custom_dve(
                    RSQRT_NR, out=flat3(r), in0=flat3(s1),
                    in1=flat3(seedb.bitcast(f32)), s0=0.5, s1=1.5, imm2=0.0,
                )

                x11 = spool.tile([128, nb, N_EQ_CH], bf16, tag="x11")
                nc.vector._custom_dve(
                    MUL_SUB1, out=flat3(x11), in0=flat3(s1), in1=flat3(r),
                    s0=0.0, s1=0.0, imm2=0.0,
                )
                t["r"] = r
                t["x11"] = x11

            def issue_T(t):
                """x11 transposes on PE (f32 PSUM) + DVE 2x copy to SBUF."""
                nb = t["nb"]
                R = nb * 128
                x11 = t["x11"]
                x11T = tpool.tile([N_EQ_CH, R], bf16, tag="x11T")
                for cb0 in range(0, nb, CHUNK_BLOCKS):
                    cnb = min(CHUNK_BLOCKS, nb - cb0)
                    Rc = cnb * 128
                    TPb = ps_tp.tile([N_EQ_CH, Rc], bf16, tag="tp")
                    for j in range(cnb):
                        nc.tensor.transpose(
                            TPb[:, j * 128 : (j + 1) * 128],
                            x11[:, cb0 + j, :], ident,
                        )
                    nc.scalar.activation(
                        out=x11T[:, cb0 * 128 : cb0 * 128 + Rc], in_=TPb,
                        func=AF.Identity)
                t["x11T"] = x11T

            def issue_L1(t):
                """x2 (GP), H1/H2 matmuls + sq1, Q1."""
                nb = t["nb"]
                R = nb * 128
                Xe, X10T, r, x11T = t["Xe"], t["X10T"], t["r"], t["x11T"]
                O = opool.tile([128, nb, DIM], bf16, tag="O")
                t["O"] = O

                for (nch, w, eqoff, choff) in SEGS:
                    rbc = (
                        r[:, :, choff : choff + nch]
                        .unsqueeze(-1)
                        .broadcast_to((128, nb, nch, w))
                    )
                    nc.gpsimd.tensor_tensor(
                        out=O[:, :, N_INV + eqoff : N_INV + eqoff + nch * w].rearrange(
                            "p b (c t) -> p b c t", t=w
                        ),
                        in0=Xe[:, :, eqoff : eqoff + nch * w].rearrange(
                            "p b (c t) -> p b c t", t=w
                        ),
                        in1=rbc,
                        op=ALU.mult,
                    )

                chunks = []
                for cb0 in range(0, nb, CHUNK_BLOCKS):
                    cnb = min(CHUNK_BLOCKS, nb - cb0)
                    chunks.append((cb0, cnb, cnb * 128))
                t["chunks"] = chunks

                sq1 = cpool.tile([128, R], bf16, tag="sq1")
                h2s = {}
                q1s = {}
                for ci, (cb0, cnb, Rc) in enumerate(chunks):
                    sl = slice(cb0 * 128, cb0 * 128 + Rc)
                    H1 = ps_mm.tile([128, Rc], f32, tag="mm")
                    mm512(H1, w1a, X10T[:, sl], True, False)
                    mm512(H1, w1b, x11T[:, sl], False, True)
                    nc.scalar.activation(out=sq1[:, sl], in_=H1, func=AF.Square)
                    H2 = ps_mm.tile([128, Rc], f32, tag="mm")
                    mm512(H2, w12a, X10T[:, sl], True, False)
                    mm512(H2, w12b, x11T[:, sl], False, True)
                    h2s[ci] = H2
                    Q1 = ps_mm.tile([128, Rc], f32, tag="mm")
                    mm512(Q1, onesd, sq1[:, sl], True, True)
                    q1s[ci] = Q1
                t["h2s"] = h2s
                t["q1s"] = q1s

            def issue_L2(t):
                """LN1 stats+apply, silu, center, LN2 stats+apply (per macro)."""
                nb = t["nb"]
                R = nb * 128
                chunks, h2s, q1s = t["chunks"], t["h2s"], t["q1s"]

                av = cpool.tile([128, R], bf16, tag="av")
                for ci, (cb0, cnb, Rc) in enumerate(chunks):
                    sl = slice(cb0 * 128, cb0 * 128 + Rc)
                    Q1 = q1s[ci]
                    sd1 = cpool.tile([128, Rc], i32, tag="sd1")
                    nc.scalar.activation(out=sd1, in_=Q1.bitcast(i32),
                                         func=AF.Identity, scale=-0.5, bias=MAGICF)
                    rstd1 = cpool.tile([128, Rc], f32, tag="rstd1")
                    nc.vector._custom_dve(
                        RSQRT_NR, out=rstd1, in0=Q1, in1=sd1.bitcast(f32),
                        s0=0.5, s1=1.5, imm2=float(EPS),
                    )
                    nc.vector.tensor_mul(av[:, sl], h2s[ci], rstd1)
                a2 = cpool.tile([128, R], bf16, tag="a2")
                nc.scalar.activation(out=a2, in_=av, func=AF.Silu, bias=b2c)

                hn2 = cpool.tile([128, R], bf16, tag="hn2")
                for ci, (cb0, cnb, Rc) in enumerate(chunks):
                    sl = slice(cb0 * 128, cb0 * 128 + Rc)
                    AC = ps_mm.tile([128, Rc], f32, tag="mm")
                    mm512(AC, cmat, a2[:, sl], True, True)
                    sq2 = cpool.tile([128, Rc], bf16, tag="sq2")
                    nc.scalar.activation(out=sq2, in_=AC, func=AF.Square)
                    Q2 = ps_mm.tile([128, Rc], f32, tag="mm")
                    mm512(Q2, onesd, sq2, True, True)
                    sd2 = cpool.tile([128, Rc], i32, tag="sd2")
                    nc.scalar.activation(out=sd2, in_=Q2.bitcast(i32),
                                         func=AF.Identity, scale=-0.5, bias=MAGICF)
                    rstd2 = cpool.tile([128, Rc], f32, tag="rstd2")
                    nc.vector._custom_dve(
                        RSQRT_NR, out=rstd2, in0=Q2, in1=sd2.bitcast(f32),
                        s0=0.5, s1=1.5, imm2=float(EPS),
                    )
                    nc.vector.tensor_mul(hn2[:, sl], AC, rstd2)
                t["hn2"] = hn2

            def issue_L3(t):
                """M3 (flip matmuls), out copy, store."""
                nb, ov, O = t["nb"], t["ov"], t["O"]
                hn2 = t["hn2"]
                for cb0, cnb, Rc in t["chunks"]:
                    H3n = ps_mm.tile([128, Rc], f32, tag="mm")
                    for j in range(cnb):
                        jj = (cb0 + j) * 128
                        nc.tensor.matmul(
                            H3n[:, j * 128 : (j + 1) * 128],
                            hn2[:, jj : jj + 128], w3p,
                            start=True, stop=True,
                            skip_group_check=True,
                        )
                    nc.vector.tensor_copy(
                        O[:, cb0 : cb0 + cnb, 0:N_INV],
                        H3n.rearrange("p (b j) -> p b j", j=128),
                    )
                nc.sync.dma_start(out=ov, in_=O)

            # ---- two-stage skewed software pipeline ----
            offs = []
            row0 = 0
            for nb in MACROS:
                offs.append((row0, nb))
                row0 += nb * 128
            states = {}
            nmac = len(offs)
            states[0] = issue_load(*offs[0])
            for i in range(nmac + 2):
                if i + 1 < nmac:
                    states[i + 1] = issue_load(*offs[i + 1])
                if i < nmac:
                    issue_eq(states[i])
                if 0 <= i - 1 < nmac:
                    issue_T(states[i - 1])
                    issue_L1(states[i - 1])
                if 0 <= i - 2 < nmac:
                    issue_L2(states[i - 2])
                    issue_L3(states[i - 2])
                    del states[i - 2]

    nc.finalize()
    return nc


_NC_CACHE = {}


def _get_nc():
    if "nc" not in _NC_CACHE:
        _NC_CACHE["nc"] = _build_nc()
    return _NC_CACHE["nc"]


# --------------------------------------------------------------- host driver --
def _bf16(a):
    import ml_dtypes
    return np.asarray(a).astype(ml_dtypes.bfloat16)


def _prep_weights(w1, g1, beta1, w2, b2, g2, beta2, w3, b3):
    C = np.eye(128, dtype=np.float64) - 1.0 / 128.0
    w1p = w1.astype(np.float64) @ C                       # [240,128]
    w2p = (g1.astype(np.float64)[:, None] * w2.astype(np.float64))
    b2c = beta1.astype(np.float64) @ w2.astype(np.float64) + b2.astype(np.float64)
    W12 = w1p @ w2p
    w3p = (g2.astype(np.float64)[:, None] * w3.astype(np.float64))
    b3c = beta2.astype(np.float64) @ w3.astype(np.float64) + b3.astype(np.float64)

    return {
        "w1a": _bf16(w1p[0:128]),
        "w1b": _bf16(w1p[128:240]),
        "w12a": _bf16(W12[0:128]),
        "w12b": _bf16(W12[128:240]),
        "w3p": _bf16(w3p),
        "cmat": _bf16(C),
        "onesd": _bf16(np.full((128, 128), 1.0 / 128.0)),
        "ident": _bf16(np.eye(128)),
        "b2c": b2c.astype(np.float32).reshape(128, 1),
    }, b3c


def _np_reference(ten, w1, g1, beta1, w2, b2, g2, beta2, w3, b3):
    """Pure-numpy fallback (used only if inputs are structurally unexpected)."""
    x10 = ten[:, :N_INV]
    eq = ten[:, N_INV:]
    sumsq = np.zeros((ten.shape[0], N_EQ_CH), np.float32)
    for (nch, w, eqoff, choff) in SEGS:
        sumsq[:, choff:choff + nch] = (
            (eq[:, eqoff:eqoff + nch * w].reshape(-1, nch, w) ** 2).sum(-1)
        )
    d = np.sqrt(sumsq + 1.0)
    x11 = d - 1.0
    x1 = np.concatenate([x10, x11], 1)
    seg = np.concatenate([np.repeat(np.arange(nch) + choff, w)
                          for (nch, w, eqoff, choff) in SEGS])
    x2 = eq / d[:, seg]

    def ln(h, g, b):
        mu = h.mean(-1, keepdims=True)
        var = h.var(-1, keepdims=True)
        return (h - mu) / np.sqrt(var + EPS) * g + b

    h = x1 @ w1
    h = ln(h, g1, beta1)
    h = h @ w2 + b2
    h = h * (1.0 / (1.0 + np.exp(-h)))
    h = ln(h, g2, beta2)
    h = h @ w3 + b3
    return np.concatenate([h, x2], 1).astype(np.float32)


def _host_inputs(ten_f32, wmap):
    """Build the 8 per-core input dicts from the full [N, DIM] f32 array."""
    import ml_dtypes
    xpad = np.zeros((NPAD, DIM), dtype=ml_dtypes.bfloat16)
    xpad[:N] = ten_f32.astype(ml_dtypes.bfloat16)
    shards = xpad.reshape(N_CORES, ROWS_PER_CORE, DIM)
    x10t_full = np.ascontiguousarray(xpad[:, 0:N_INV].T)   # [128, NPAD] bf16
    in_maps = []
    for c in range(N_CORES):
        sl = slice(c * ROWS_PER_CORE, (c + 1) * ROWS_PER_CORE)
        in_maps.append(dict(
            wmap,
            x=np.ascontiguousarray(shards[c]),
            x10t=np.ascontiguousarray(x10t_full[:, sl]),
        ))
    return in_maps


def kernel(ten, rep_layout, w1, g1, beta1, w2, b2, g2, beta2, w3, b3):
    ten = np.asarray(ten, dtype=np.float32)
    args = [np.asarray(a) for a in (w1, g1, beta1, w2, b2, g2, beta2, w3, b3)]
    w1, g1, beta1, w2, b2, g2, beta2, w3, b3 = [a.astype(np.float32) for a in args]

    if not np.array_equal(np.asarray(rep_layout).astype(np.int64), _EXPECTED_REP):
        return _np_reference(ten, w1, g1, beta1, w2, b2, g2, beta2, w3, b3)

    wmap, b3c = _prep_weights(w1, g1, beta1, w2, b2, g2, beta2, w3, b3)
    if not np.allclose(b3c, 0.0, atol=1e-12):
        return _np_reference(ten, w1, g1, beta1, w2, b2, g2, beta2, w3, b3)

    nc = _get_nc()
    in_maps = _host_inputs(ten, wmap)
    last_err = None
    for _attempt in range(3):
        try:
            res = run_bass_kernel_spmd(nc, in_maps, list(range(N_CORES))).results
            break
        except Exception as e:  # transient device-unrecoverable errors
            last_err = e
            import time as _time
            _time.sleep(10)
    else:
        raise last_err
    outp = np.concatenate([res[c]["out"] for c in range(N_CORES)], axis=0)
    return np.ascontiguousarray(outp[:N].astype(np.float32))


# revision 27
# speedup vs baseline: 1.0006x; 1.0006x over previous
"""Trainium2 Bass kernel for nn_EvMLP (segment_reduce EvNorm + invariant MLP).

Self-contained: hardcodes shapes/sharding. Accepts FULL inputs, returns FULL
output; shards the node dim N across 8 NeuronCores (pure data parallel).

v3 design (on top of the bf16 v2 baseline):
  - x10T supplied pre-transposed by the HOST (plain contiguous DMA load;
    the HW DMA-transpose of [R,128] made 256B packets and throttled DMA)
  - all eq squares on ACT (one Square op per macro); GP keeps only x2
  - x11 written into a 128-wide padded tile; x11T produced by an
    SBUF->SBUF DMA transpose (PE transposes + ACT PSUM-copies eliminated);
    w1b/W12b zero-padded to [128,128] so junk partitions multiply by 0
  - LN stats batched per macro: Q1/Q2 are [128, nb*128] f32 PSUM tiles
    (2 banks), seed + NR custom ops run at FD=1024 (overhead amortized)
  - b3 rank-1 matmul dropped (b3c==0 for this problem; np fallback guards)
"""
import sys

sys.path.insert(0, "/opt/trn_rl_repo")

import numpy as np

import concourse.bass as bass
import concourse.bacc as bacc
import concourse.tile as tile
from concourse import mybir
from concourse.bass_utils import run_bass_kernel_spmd

f32 = mybir.dt.float32
i32 = mybir.dt.int32
bf16 = mybir.dt.bfloat16

# ---------------------------------------------------------------- constants --
N = 100000
DIM = 592
N_INV = 128
N_EQ_CH = 112
N_EQ = 464
EPS = 1e-5
N_CORES = 8
BLOCKS_PER_CORE = 98                      # 98*128 = 12544 rows/core
ROWS_PER_CORE = BLOCKS_PER_CORE * 128
NPAD = N_CORES * ROWS_PER_CORE            # 100352
MACROS = [8] * 12 + [2]                   # blocks per macro-tile (sum 98)
CHUNK_BLOCKS = 4                          # rows per MLP chunk = 512
MAGIC = 0x5F3759DF
MAGICF = float(MAGIC)

# segment groups: (n_channels, width, eq column offset, channel offset)
SEGS = [(64, 3, 0, 0), (32, 5, 192, 64), (16, 7, 352, 96)]

_EXPECTED_REP = np.concatenate(
    [np.repeat(np.arange(m) + off, 2 * l + 1)
     for l, (m, off) in enumerate([(128, 0), (64, 128), (32, 192), (16, 224)])]
)

# ------------------------------------------------------------- custom DVE op --
from concourse.dve_spec import Spec, Src0, Src1, C0, C1, C2, lower
from concourse.dve_uop import DveOpSpec
import concourse.dve_ops as dve_ops
from concourse.dve_ops import DveOp

# Newton rsqrt step: out = y*(C1 - C0*((v+C2)*y*y));  in0=v, in1=y
_nr_body = Src1 * (C1 - ((Src0 + C2) * (Src1 * Src1)) * C0)


def _nr_ref(in0, in1, s0, s1, imm2):
    y = in1.astype(np.float32)
    v = in0.astype(np.float32)
    return (y * (np.float32(s1) - ((v + np.float32(imm2)) * y * y) * np.float32(s0))
            ).astype(np.float32)


def _register(name, spec):
    if name in dve_ops._SUB_OPCODE_FOR_NAME:
        for op in dve_ops.OPS:
            if op.name == name:
                return op
    shas = {}
    row = 1 + len(dve_ops.OPS)
    for ver in ("v3", "v4"):
        s = DveOpSpec(name=name, opcode=row, uops=lower(spec, ver=ver), rd1_en=True)
        shas[ver] = s.sha(ver)
    op = DveOp(name, spec, subdim=False, uops_sha=shas)
    dve_ops.OPS.append(op)
    dve_ops._SUB_OPCODE_FOR_NAME[name] = row
    dve_ops.CUSTOM_DVE_SPECS[name] = spec
    return op


RSQRT_NR = _register("ANT_RSQRT_NR2", Spec(body=_nr_body, reference=_nr_ref))


def _make_mulsub1():
    from concourse.dve_spec import One
    return _register(
        "ANT_MUL_SUB1",
        Spec(
            body=(Src0 * Src1) - One,
            reference=lambda in0, in1, s0, s1, imm2: (
                in0.astype(np.float32) * in1 - np.float32(1.0)
            ).astype(np.float32),
        ),
    )


MUL_SUB1 = _make_mulsub1()


# ------------------------------------------------------------ kernel builder --
def _build_nc():
    nc = bacc.Bacc()

    x = nc.dram_tensor("x", [ROWS_PER_CORE, DIM], bf16, kind="ExternalInput")
    x10t_d = nc.dram_tensor("x10t", [128, ROWS_PER_CORE], bf16, kind="ExternalInput")
    out = nc.dram_tensor("out", [ROWS_PER_CORE, DIM], bf16, kind="ExternalOutput")
    w1a_d = nc.dram_tensor("w1a", [128, 128], bf16, kind="ExternalInput")
    w1b_d = nc.dram_tensor("w1b", [112, 128], bf16, kind="ExternalInput")
    w12a_d = nc.dram_tensor("w12a", [128, 128], bf16, kind="ExternalInput")
    w12b_d = nc.dram_tensor("w12b", [112, 128], bf16, kind="ExternalInput")
    w3_d = nc.dram_tensor("w3p", [128, 128], bf16, kind="ExternalInput")
    cmat_d = nc.dram_tensor("cmat", [128, 128], bf16, kind="ExternalInput")
    onesd_d = nc.dram_tensor("onesd", [128, 128], bf16, kind="ExternalInput")
    ident_d = nc.dram_tensor("ident", [128, 128], bf16, kind="ExternalInput")
    b2_d = nc.dram_tensor("b2c", [128, 1], f32, kind="ExternalInput")

    # extra float consts used as activation bias (register like Bass.__init__)
    for _v in (MAGICF, float(EPS), 1.0):
        _t = nc.alloc_sbuf_tensor(f"const-f32-{_v}", [128, 1], f32)
        nc.gpsimd.memset(_t.ap(), _v)
        nc.const_aps.aps[(f32, _v)] = _t.ap()
    nc.all_engine_barrier()

    AF = mybir.ActivationFunctionType
    ALU = mybir.AluOpType
    AX = mybir.AxisListType

    from contextlib import ExitStack

    with tile.TileContext(nc) as tc:
        with ExitStack() as ctx:
            wpool = ctx.enter_context(tc.tile_pool(name="w", bufs=1))
            xpool = ctx.enter_context(tc.tile_pool(name="xp", bufs=3))
            opool = ctx.enter_context(tc.tile_pool(name="op", bufs=3))
            epool = ctx.enter_context(tc.tile_pool(name="ep", bufs=3))
            spool = ctx.enter_context(tc.tile_pool(name="sp", bufs=4))
            cpool = ctx.enter_context(tc.tile_pool(name="cp", bufs=3))
            tpool = ctx.enter_context(tc.tile_pool(name="tp", bufs=2))
            ps_mm = ctx.enter_context(tc.tile_pool(name="pmm", bufs=7, space="PSUM"))
            ps_tp = ctx.enter_context(tc.tile_pool(name="ptp", bufs=1, space="PSUM"))

            def wtile(name, dram, shape, dtype):
                t = wpool.tile(shape, dtype, tag=name)
                nc.sync.dma_start(out=t, in_=dram[:, :])
                return t

            w1a = wtile("w1a", w1a_d, [128, 128], bf16)
            w1b = wtile("w1b", w1b_d, [112, 128], bf16)
            w12a = wtile("w12a", w12a_d, [128, 128], bf16)
            w12b = wtile("w12b", w12b_d, [112, 128], bf16)
            w3p = wtile("w3p", w3_d, [128, 128], bf16)
            cmat = wtile("cmat", cmat_d, [128, 128], bf16)
            onesd = wtile("onesd", onesd_d, [128, 128], bf16)
            ident = wtile("ident", ident_d, [128, 128], bf16)
            b2c = wtile("b2c", b2_d, [128, 1], f32)

            flat3 = lambda ap: ap.rearrange("p a b -> p (a b)")

            def mm512(out_ap, lhsT, rhs, start, stop, skip=False):
                F = rhs.shape[-1]
                for f0 in range(0, F, 512):
                    f1 = min(f0 + 512, F)
                    nc.tensor.matmul(
                        out_ap[:, f0:f1], lhsT, rhs[:, f0:f1],
                        start=start, stop=stop, skip_group_check=skip,
                    )

            def issue_load(row0, nb):
                """Prefetch macro tiles (issued one iteration early)."""
                R_rows = nb * 128
                xev = x[row0 : row0 + R_rows, N_INV:DIM].rearrange(
                    "(b p) d -> p b d", p=128
                )
                ov = out[row0 : row0 + R_rows, :].rearrange("(b p) d -> p b d", p=128)

                Xe = xpool.tile([128, nb, N_EQ], bf16, tag="Xe")
                nc.sync.dma_start(out=Xe, in_=xev)
                X10T = xpool.tile([128, R_rows], bf16, tag="X10T")
                nc.sync.dma_start(out=X10T, in_=x10t_d[:, row0 : row0 + R_rows])
                return dict(nb=nb, ov=ov, Xe=Xe, X10T=X10T)

            def issue_eq(t):
                """eq chain through x11 (inputs prefetched last iteration)."""
                nb, Xe = t["nb"], t["Xe"]
                eq2 = epool.tile([128, nb, N_EQ], bf16, tag="eq2")
                for (nch, w, eqoff, choff) in SEGS:
                    e0, e1 = eqoff, eqoff + nch * w
                    nc.scalar.activation(
                        out=eq2[:, :, e0:e1], in_=Xe[:, :, e0:e1], func=AF.Square)

                sumsq = spool.tile([128, nb, N_EQ_CH], f32, tag="sumsq")
                for (nch, w, eqoff, choff) in SEGS:
                    nc.vector.reduce_sum(
                        out=sumsq[:, :, choff : choff + nch],
                        in_=eq2[:, :, eqoff : eqoff + nch * w].rearrange(
                            "p b (c t) -> p b c t", t=w
                        ),
                        axis=AX.X,
                    )

                s1 = spool.tile([128, nb, N_EQ_CH], f32, tag="s1")
                nc.scalar.activation(out=s1, in_=sumsq, func=AF.Identity, bias=1.0)
                seedb = spool.tile([128, nb, N_EQ_CH], i32, tag="seedb")
                nc.scalar.activation(
                    out=seedb, in_=s1.bitcast(i32), func=AF.Identity,
                    scale=-0.5, bias=MAGICF,
                )
                r = spool.tile([128, nb, N_EQ_CH], f32, tag="r")
                nc.vector._custom_dve(
                    RSQRT_NR, out=flat3(r), in0=flat3(s1),
                    in1=flat3(seedb.bitcast(f32)), s0=0.5, s1=1.5, imm2=0.0,
                )

                x11 = spool.tile([128, nb, N_EQ_CH], bf16, tag="x11")
                nc.vector._custom_dve(
                    MUL_SUB1, out=flat3(x11), in0=flat3(s1), in1=flat3(r),
                    s0=0.0, s1=0.0, imm2=0.0,
                )
                t["r"] = r
                t["x11"] = x11

            def issue_T(t):
                """x11 transposes on PE (f32 PSUM) + DVE 2x copy to SBUF."""
                nb = t["nb"]
                R = nb * 128
                x11 = t["x11"]
                x11T = tpool.tile([N_EQ_CH, R], bf16, tag="x11T")
                for cb0 in range(0, nb, CHUNK_BLOCKS):
                    cnb = min(CHUNK_BLOCKS, nb - cb0)
                    Rc = cnb * 128
                    TPb = ps_tp.tile([N_EQ_CH, Rc], bf16, tag="tp")
                    for j in range(cnb):
                        nc.tensor.transpose(
                            TPb[:, j * 128 : (j + 1) * 128],
                            x11[:, cb0 + j, :], ident,
                        )
                    nc.scalar.activation(
                        out=x11T[:, cb0 * 128 : cb0 * 128 + Rc], in_=TPb,
                        func=AF.Identity)
                t["x11T"] = x11T

            def issue_L1(t):
                """x2 (GP), H1/H2 matmuls + sq1, Q1."""
                nb = t["nb"]
                R = nb * 128
                Xe, X10T, r, x11T = t["Xe"], t["X10T"], t["r"], t["x11T"]
                O = opool.tile([128, nb, DIM], bf16, tag="O")
                t["O"] = O

                for (nch, w, eqoff, choff) in SEGS:
                    rbc = (
                        r[:, :, choff : choff + nch]
                        .unsqueeze(-1)
                        .broadcast_to((128, nb, nch, w))
                    )
                    nc.gpsimd.tensor_tensor(
                        out=O[:, :, N_INV + eqoff : N_INV + eqoff + nch * w].rearrange(
                            "p b (c t) -> p b c t", t=w
                        ),
                        in0=Xe[:, :, eqoff : eqoff + nch * w].rearrange(
                            "p b (c t) -> p b c t", t=w
                        ),
                        in1=rbc,
                        op=ALU.mult,
                    )

                chunks = []
                for cb0 in range(0, nb, CHUNK_BLOCKS):
                    cnb = min(CHUNK_BLOCKS, nb - cb0)
                    chunks.append((cb0, cnb, cnb * 128))
                t["chunks"] = chunks

                sq1 = cpool.tile([128, R], bf16, tag="sq1")
                h2s = {}
                q1s = {}
                for ci, (cb0, cnb, Rc) in enumerate(chunks):
                    sl = slice(cb0 * 128, cb0 * 128 + Rc)
                    H1 = ps_mm.tile([128, Rc], f32, tag="mm")
                    mm512(H1, w1a, X10T[:, sl], True, False)
                    mm512(H1, w1b, x11T[:, sl], False, True)
                    nc.scalar.activation(out=sq1[:, sl], in_=H1, func=AF.Square)
                    H2 = ps_mm.tile([128, Rc], f32, tag="mm")
                    mm512(H2, w12a, X10T[:, sl], True, False)
                    mm512(H2, w12b, x11T[:, sl], False, True)
                    h2s[ci] = H2
                    Q1 = ps_mm.tile([128, Rc], f32, tag="mm")
                    mm512(Q1, onesd, sq1[:, sl], True, True)
                    q1s[ci] = Q1
                t["h2s"] = h2s
                t["q1s"] = q1s

            def issue_L2(t):
                """LN1 stats+apply, silu, center, LN2 stats+apply (per macro)."""
                nb = t["nb"]
                R = nb * 128
                chunks, h2s, q1s = t["chunks"], t["h2s"], t["q1s"]

                av = cpool.tile([128, R], bf16, tag="av")
                for ci, (cb0, cnb, Rc) in enumerate(chunks):
                    sl = slice(cb0 * 128, cb0 * 128 + Rc)
                    Q1 = q1s[ci]
                    sd1 = cpool.tile([128, Rc], i32, tag="sd1")
                    nc.scalar.activation(out=sd1, in_=Q1.bitcast(i32),
                                         func=AF.Identity, scale=-0.5, bias=MAGICF)
                    rstd1 = cpool.tile([128, Rc], f32, tag="rstd1")
                    nc.vector._custom_dve(
                        RSQRT_NR, out=rstd1, in0=Q1, in1=sd1.bitcast(f32),
                        s0=0.5, s1=1.5, imm2=float(EPS),
                    )
                    nc.vector.tensor_mul(av[:, sl], h2s[ci], rstd1)
                a2 = cpool.tile([128, R], bf16, tag="a2")
                nc.scalar.activation(out=a2, in_=av, func=AF.Silu, bias=b2c)

                hn2 = cpool.tile([128, R], bf16, tag="hn2")
                for ci, (cb0, cnb, Rc) in enumerate(chunks):
                    sl = slice(cb0 * 128, cb0 * 128 + Rc)
                    AC = ps_mm.tile([128, Rc], f32, tag="mm")
                    mm512(AC, cmat, a2[:, sl], True, True)
                    sq2 = cpool.tile([128, Rc], bf16, tag="sq2")
                    nc.scalar.activation(out=sq2, in_=AC, func=AF.Square)
                    Q2 = ps_mm.tile([128, Rc], f32, tag="mm")
                    mm512(Q2, onesd, sq2, True, True)
                    sd2 = cpool.tile([128, Rc], i32, tag="sd2")
                    nc.scalar.activation(out=sd2, in_=Q2.bitcast(i32),
                                         func=AF.Identity, scale=-0.5, bias=MAGICF)
                    rstd2 = cpool.tile([128, Rc], f32, tag="rstd2")
                    nc.vector._custom_dve(
                        RSQRT_NR, out=rstd2, in0=Q2, in1=sd2.bitcast(f32),
                        s0=0.5, s1=1.5, imm2=float(EPS),
                    )
                    nc.vector.tensor_mul(hn2[:, sl], AC, rstd2)
                t["hn2"] = hn2

            def issue_L3(t):
                """M3 (flip matmuls), out copy, store."""
                nb, ov, O = t["nb"], t["ov"], t["O"]
                hn2 = t["hn2"]
                for cb0, cnb, Rc in t["chunks"]:
                    H3n = ps_mm.tile([128, Rc], f32, tag="mm")
                    for j in range(cnb):
                        jj = (cb0 + j) * 128
                        nc.tensor.matmul(
                            H3n[:, j * 128 : (j + 1) * 128],
                            hn2[:, jj : jj + 128], w3p,
                            start=True, stop=True,
                            skip_group_check=True,
                        )
                    nc.vector.tensor_copy(
                        O[:, cb0 : cb0 + cnb, 0:N_INV],
                        H3n.rearrange("p (b j) -> p b j", j=128),
                    )
                nc.sync.dma_start(out=ov, in_=O)

            # ---- two-stage skewed software pipeline ----
            offs = []
            row0 = 0
            for nb in MACROS:
                offs.append((row0, nb))
                row0 += nb * 128
            states = {}
            nmac = len(offs)
            states[0] = issue_load(*offs[0])
            for i in range(nmac + 2):
                if i + 1 < nmac:
                    states[i + 1] = issue_load(*offs[i + 1])
                if i < nmac:
                    issue_eq(states[i])
                if 0 <= i - 1 < nmac:
                    issue_T(states[i - 1])
                    issue_L1(states[i - 1])
                if 0 <= i - 2 < nmac:
                    issue_L2(states[i - 2])
                    issue_L3(states[i - 2])
                    del states[i - 2]

    nc.finalize()
    return nc


_NC_CACHE = {}


def _get_nc():
    if "nc" not in _NC_CACHE:
        _NC_CACHE["nc"] = _build_nc()
    return _NC_CACHE["nc"]


# --------------------------------------------------------------- host driver --
def _bf16(a):
    import ml_dtypes
    return np.asarray(a).astype(ml_dtypes.bfloat16)


def _prep_weights(w1, g1, beta1, w2, b2, g2, beta2, w3, b3):
    C = np.eye(128, dtype=np.float64) - 1.0 / 128.0
    w1p = w1.astype(np.float64) @ C                       # [240,128]
    w2p = (g1.astype(np.float64)[:, None] * w2.astype(np.float64))
    b2c = beta1.astype(np.float64) @ w2.astype(np.float64) + b2.astype(np.float64)
    W12 = w1p @ w2p
    w3p = (g2.astype(np.float64)[:, None] * w3.astype(np.float64))
    b3c = beta2.astype(np.float64) @ w3.astype(np.float64) + b3.astype(np.float64)

    return {
        "w1a": _bf16(w1p[0:128]),
        "w1b": _bf16(w1p[128:240]),
        "w12a": _bf16(W12[0:128]),
        "w12b": _bf16(W12[128:240]),
        "w3p": _bf16(w3p),
        "cmat": _bf16(C),
        "onesd": _bf16(np.full((128, 128), 1.0 / 128.0)),
        "ident": _bf16(np.eye(128)),
        "b2c": b2c.astype(np.float32).reshape(128, 1),
    }, b3c


def _np_reference(ten, w1, g1, beta1, w2, b2, g2, beta2, w3, b3):
    """Pure-numpy fallback (used only if inputs are structurally unexpected)."""
    x10 = ten[:, :N_INV]
    eq = ten[:, N_INV:]
    sumsq = np.zeros((ten.shape[0], N_EQ_CH), np.float32)
    for (nch, w, eqoff, choff) in SEGS:
        sumsq[:, choff:choff + nch] = (
            (eq[:, eqoff:eqoff + nch * w].reshape(-1, nch, w) ** 2).sum(-1)
        )
    d = np.sqrt(sumsq + 1.0)
    x11 = d - 1.0
    x1 = np.concatenate([x10, x11], 1)
    seg = np.concatenate([np.repeat(np.arange(nch) + choff, w)
                          for (nch, w, eqoff, choff) in SEGS])
    x2 = eq / d[:, seg]

    def ln(h, g, b):
        mu = h.mean(-1, keepdims=True)
        var = h.var(-1, keepdims=True)
        return (h - mu) / np.sqrt(var + EPS) * g + b

    h = x1 @ w1
    h = ln(h, g1, beta1)
    h = h @ w2 + b2
    h = h * (1.0 / (1.0 + np.exp(-h)))
    h = ln(h, g2, beta2)
    h = h @ w3 + b3
    return np.concatenate([h, x2], 1).astype(np.float32)


def _host_inputs(ten_f32, wmap):
    """Build the 8 per-core input dicts from the full [N, DIM] f32 array."""
    import ml_dtypes
    xpad = np.zeros((NPAD, DIM), dtype=ml_dtypes.bfloat16)
    xpad[:N] = ten_f32.astype(ml_dtypes.bfloat16)
    shards = xpad.reshape(N_CORES, ROWS_PER_CORE, DIM)
    x10t_full = np.ascontiguousarray(xpad[:, 0:N_INV].T)   # [128, NPAD] bf16
    in_maps = []
    for c in range(N_CORES):
        sl = slice(c * ROWS_PER_CORE, (c + 1) * ROWS_PER_CORE)
        in_maps.append(dict(
            wmap,
            x=np.ascontiguousarray(shards[c]),
            x10t=np.ascontiguousarray(x10t_full[:, sl]),
        ))
    return in_maps


def kernel(ten, rep_layout, w1, g1, beta1, w2, b2, g2, beta2, w3, b3):
    ten = np.asarray(ten, dtype=np.float32)
    args = [np.asarray(a) for a in (w1, g1, beta1, w2, b2, g2, beta2, w3, b3)]
    w1, g1, beta1, w2, b2, g2, beta2, w3, b3 = [a.astype(np.float32) for a in args]

    if not np.array_equal(np.asarray(rep_layout).astype(np.int64), _EXPECTED_REP):
        return _np_reference(ten, w1, g1, beta1, w2, b2, g2, beta2, w3, b3)

    wmap, b3c = _prep_weights(w1, g1, beta1, w2, b2, g2, beta2, w3, b3)
    if not np.allclose(b3c, 0.0, atol=1e-12):
        return _np_reference(ten, w1, g1, beta1, w2, b2, g2, beta2, w3, b3)

    nc = _get_nc()
    in_maps = _host_inputs(ten, wmap)
    last_err = None
    for _attempt in range(3):
        try:
            res = run_bass_kernel_spmd(nc, in_maps, list(range(N_CORES))).results
            break
        except Exception as e:  # transient device-unrecoverable errors
            last_err = e
            import time as _time
            _time.sleep(10)
    else:
        raise last_err
    outp = np.concatenate([res[c]["out"] for c in range(N_CORES)], axis=0)
    return np.ascontiguousarray(outp[:N].astype(np.float32))
